# revision 33
# baseline (speedup 1.0000x reference)
"""Trainium2 Bass kernel for nn_ModalityMoERouter (expert-choice MoE routing).

Contract: kernel(**inputs) takes the FULL inputs from reference.setup_inputs()
and returns (dispatch, combine), each (16, 8192, 16) float32.

Sharding: data-parallel over batch B=16 across 8 NeuronCores (2 batches/core);
gate weights and expert centers replicated. The global mean(dists) scalar is
computed with one AllReduce (overlapped with the gate MLP).

Design (v2, ~4x faster than v1):
 - tokens are pre-transposed on the host to [BPC, D, N] so the W1 matmul
   consumes them directly (no PE transposes, no PSUM->SBUF copies).
 - all large matmuls run as float32r (full fp32 operands, 1 cycle/row on the
   PE when the free dim is >= 256, vs 4 cycles/row for plain fp32).
 - distances via one fused matmul pair: dist^2 = S1 @ xyz + S2 @ xyz^2 + |c|^2
   with block-diagonal stationaries (host-precomputed), sqrt + one Heron step.
 - expert-choice top-k (k=1024 of N=8192 per (batch, expert)) via branch-free
   bisection on the count of logits above a candidate threshold. The count's
   cross-partition reduction uses DVE partition-offset tree-adds (no PE in the
   chain), so bisect(b0) overlaps MLP(b1) without head-of-line blocking the
   tensor engine FIFO.
 - outputs are written in the on-chip [128, 2048] layout (p = g*16+e,
   f = b*1024 + blk*512 + t with token n = (blk*8+g)*512 + t) and unscrambled
   on the host.
 - the hard-cap + redistribution step in the reference is exactly a no-op:
   dispatch after the routing floor is <= 0.4*sigmoid + 0.0375 <= 0.4375,
   while cap >= 0.5, so excess == 0 bitwise. It is therefore skipped (t unused).
"""

import numpy as np

B = 16
N = 8192
D = 512
H = 256
E = 16
N_CORES = 8
BPC = B // N_CORES
NT = N // 512               # 16 tiles of 512 tokens per batch
KSEL = N * 2 // E           # 1024
ALPHA = min(min(0.05, 0.15 / 4) * E, 1.0)
DSCALE = 1.0 - ALPHA        # 0.4
DFLOOR = ALPHA / E          # 0.0375
N_ITER = 18
W0 = 8.0                    # bisection window [-4, 4]; theta in [-1.91, 0.52]
_MM_F32R = False            # use float32r (1 cyc/row) for big matmuls
_DEBUG = False

_prog_cache = {}


def _build(debug=False):
    import concourse.bacc as bacc
    import concourse.mybir as mybir
    import concourse.tile as tile

    F32 = mybir.dt.float32
    F32R = mybir.dt.float32r
    AO = mybir.AluOpType
    AF = mybir.ActivationFunctionType

    MMDT = F32R if _MM_F32R else F32

    def r(ap):
        return ap

    nc = bacc.Bacc("TRN2", num_devices=N_CORES)

    F16 = mybir.dt.float16
    tokh_d = nc.dram_tensor("tokh", [BPC, D, N], F16, kind="ExternalInput")
    tokl_d = nc.dram_tensor("tokl", [BPC, D, N], F16, kind="ExternalInput")
    xyzT_d = nc.dram_tensor("xyzT", [BPC, 3, N], MMDT, kind="ExternalInput")
    xyzh_d = nc.dram_tensor("xyzh", [BPC, 3, N], F16, kind="ExternalInput")
    xyzl_d = nc.dram_tensor("xyzl", [BPC, 3, N], F16, kind="ExternalInput")
    W1h_d = nc.dram_tensor("W1h", [D + 3, H], F16, kind="ExternalInput")
    W1l_d = nc.dram_tensor("W1l", [D + 3, H], F16, kind="ExternalInput")
    W1x9_d = nc.dram_tensor("W1x9", [9, H], F16, kind="ExternalInput")
    b1_d = nc.dram_tensor("b1", [H], F32, kind="ExternalInput")
    W2_d = nc.dram_tensor("W2", [H, E], MMDT, kind="ExternalInput")
    b2t_d = nc.dram_tensor("b2t", [E, 1], F32, kind="ExternalInput")
    S1_d = nc.dram_tensor("S1", [24, 128], MMDT, kind="ExternalInput")
    S2_d = nc.dram_tensor("S2", [24, 128], MMDT, kind="ExternalInput")
    c2rep_d = nc.dram_tensor("c2rep", [128, 1], F32, kind="ExternalInput")
    bd16_d = nc.dram_tensor("bd16", [128, 128], MMDT, kind="ExternalInput")
    m2_d = nc.dram_tensor("m2", [128, 128], F32, kind="ExternalInput")

    disp_d = nc.dram_tensor("disp", [128, 2 * 1024], MMDT, kind="ExternalOutput")
    comb_d = nc.dram_tensor("comb", [128, 2 * 1024], F32, kind="ExternalOutput")
    if debug:
        dbg_logits_d = nc.dram_tensor("dbg_logits", [128, 2048], F32,
                                      kind="ExternalOutput")
        dbg_dists_d = nc.dram_tensor("dbg_dists", [128, 2048], F32,
                                     kind="ExternalOutput")

    with tile.TileContext(nc) as tc:
        with tc.tile_pool(name="const", bufs=1) as cpool, \
             tc.tile_pool(name="big", bufs=1) as bigpool, \
             tc.tile_pool(name="tok", bufs=2) as tokp, \
             tc.tile_pool(name="work", bufs=2) as work, \
             tc.tile_pool(name="ps", bufs=3, space="PSUM") as ps, \
             tc.tile_pool(name="ps2", bufs=2, space="PSUM") as ps2, \
             tc.tile_pool(name="dram", bufs=1, space="DRAM") as dram:

            # ---- constants ----
            w1_sb = {}
            w1x_sb = {}
            for hl, W1d_ in (("h", W1h_d), ("l", W1l_d)):
                blks = []
                for kc in range(4):
                    row = []
                    for mc in range(2):
                        t = cpool.tile([128, 128], F16, tag=f"w1{hl}_{kc}_{mc}",
                                       name=f"w1{hl}_{kc}_{mc}")
                        nc.sync.dma_start(
                            out=t[:], in_=W1d_[kc * 128:(kc + 1) * 128,
                                               mc * 128:(mc + 1) * 128])
                        row.append(t)
                    blks.append(row)
                w1_sb[hl] = blks
            w1x9_sb = []
            for mc in range(2):
                t = cpool.tile([9, 128], F16, tag=f"w1x9_{mc}",
                               name=f"w1x9_{mc}")
                nc.sync.dma_start(out=t[:],
                                  in_=W1x9_d[:, mc * 128:(mc + 1) * 128])
                w1x9_sb.append(t)
            b1_sb = []
            for mc in range(2):
                t = cpool.tile([128, 1], F32, tag=f"b1_{mc}", name=f"b1_{mc}")
                nc.sync.dma_start(out=t[:],
                                  in_=b1_d[mc * 128:(mc + 1) * 128].unsqueeze(1))
                b1_sb.append(t)
            w2_sb = []
            for c in range(2):
                t = cpool.tile([128, E], MMDT, tag=f"w2_{c}", name=f"w2_{c}")
                nc.sync.dma_start(out=t[:], in_=W2_d[c * 128:(c + 1) * 128, :])
                w2_sb.append(t)
            b2t_sb = cpool.tile([E, 1], F32, tag="b2t", name="b2t")
            nc.sync.dma_start(out=b2t_sb[:], in_=b2t_d[:])
            S1_sb = cpool.tile([24, 128], MMDT, tag="S1", name="S1")
            nc.sync.dma_start(out=S1_sb[:], in_=S1_d[:])
            S2_sb = cpool.tile([24, 128], MMDT, tag="S2", name="S2")
            nc.sync.dma_start(out=S2_sb[:], in_=S2_d[:])
            c2rep_sb = cpool.tile([128, 1], F32, tag="c2rep", name="c2rep")
            nc.sync.dma_start(out=c2rep_sb[:], in_=c2rep_d[:])
            bd16_sb = cpool.tile([128, 128], MMDT, tag="bd16", name="bd16")
            nc.sync.dma_start(out=bd16_sb[:], in_=bd16_d[:])
            m2_sb = cpool.tile([128, 128], F32, tag="m2", name="m2")
            nc.sync.dma_start(out=m2_sb[:], in_=m2_d[:])
            ones_1x128 = cpool.tile([1, 128], F32, tag="o1x", name="o1x")
            nc.vector.memset(ones_1x128[:], 1.0)
            ones_128x1 = cpool.tile([128, 1], F32, tag="ox1", name="ox1")
            nc.vector.memset(ones_128x1[:], 1.0)

            # ---- persistent tiles ----
            logits_A = bigpool.tile([128, 2048], F32, tag="logits", name="logits")
            dists_A = bigpool.tile([128, 2048], F32, tag="dists", name="dists")
            mscr = [bigpool.tile([128, 1024], F32, tag=f"mscr{b}",
                                 name=f"mscr{b}") for b in range(BPC)]

            # ============ Phase A: distances + global mean =================
            rs = []
            for b in range(BPC):
                for blk in range(2):
                    off = b * 1024 + blk * 512
                    xyzg = work.tile([24, 512], MMDT, tag="xyzg", name="xyzg")
                    for c in range(3):
                        nc.sync.dma_start(
                            out=xyzg[c * 8:(c + 1) * 8, :],
                            in_=xyzT_d[b, c, blk * 4096:(blk + 1) * 4096]
                                .rearrange("(g t) -> g t", g=8))
                    xyzg2 = work.tile([24, 512], MMDT, tag="xyzg2", name="xyzg2")
                    nc.vector.tensor_tensor(out=xyzg2[:], in0=xyzg[:],
                                            in1=xyzg[:], op=AO.mult)
                    p_d = ps2.tile([128, 512], F32, tag="pd", name="p_d", bufs=1)
                    nc.tensor.matmul(p_d[:], r(S1_sb[:]), r(xyzg[:]),
                                     start=True, stop=False)
                    nc.tensor.matmul(p_d[:], r(S2_sb[:]), r(xyzg2[:]),
                                     start=False, stop=True)
                    # dist = sqrt(p_d + |c|^2), one Heron step
                    y0 = work.tile([128, 512], F32, tag="y0", name="y0")
                    nc.scalar.activation(y0[:], p_d[:], AF.Sqrt,
                                         bias=c2rep_sb[:])
                    d2 = work.tile([128, 512], F32, tag="d2", name="d2")
                    nc.vector.tensor_scalar(out=d2[:], in0=p_d[:],
                                            scalar1=c2rep_sb[:], scalar2=None,
                                            op0=AO.add)
                    rr = work.tile([128, 512], F32, tag="ry", name="ry")
                    hscr = work.tile([128, 512], F32, tag="hscr", name="hscr")
                    nc.vector.reciprocal_approx_accurate(rr[:], y0[:], hscr[:])
                    nc.vector.tensor_tensor(out=rr[:], in0=d2[:], in1=rr[:],
                                            op=AO.mult)
                    nc.vector.tensor_tensor(out=rr[:], in0=rr[:], in1=y0[:],
                                            op=AO.add)
                    rs_p = bigpool.tile([128, 1], F32, tag=f"rs{b}{blk}",
                                        name=f"rs{b}{blk}")
                    nc.vector.tensor_scalar(out=dists_A[:, off:off + 512],
                                            in0=rr[:], scalar1=0.5, scalar2=0.0,
                                            op0=AO.mult, op1=AO.add,
                                            accum_out=rs_p[:])
                    rs.append(rs_p)

            rsum = work.tile([128, 1], F32, tag="rsum", name="rsum")
            nc.vector.tensor_tensor(out=rsum[:], in0=rs[0][:], in1=rs[1][:],
                                    op=AO.add)
            nc.vector.tensor_tensor(out=rsum[:], in0=rsum[:], in1=rs[2][:],
                                    op=AO.add)
            nc.vector.tensor_tensor(out=rsum[:], in0=rsum[:], in1=rs[3][:],
                                    op=AO.add)
            p_tot = ps2.tile([128, 1], F32, tag="xt", name="p_tot", bufs=1)
            nc.tensor.matmul(p_tot[0:1, :], ones_128x1[:], rsum[:], start=True,
                             stop=True)
            s_tot = work.tile([1, 1], F32, tag="stot", name="stot")
            nc.vector.tensor_copy(s_tot[:], p_tot[0:1, :])
            p_bc = ps2.tile([128, 1], F32, tag="xt", name="p_bc", bufs=1)
            nc.tensor.matmul(p_bc[:], ones_1x128[:], s_tot[:], start=True,
                             stop=True)
            sb_bc = work.tile([128, 1], F32, tag="sbbc", name="sbbc")
            nc.vector.tensor_copy(sb_bc[:], p_bc[:])
            cc_in = dram.tile([128, 1], F32)
            cc_out = dram.tile([128, 1], F32, addr_space="Shared")
            nc.sync.dma_start(out=cc_in[:], in_=sb_bc[:])
            nc.gpsimd.collective_compute(
                "AllReduce", AO.add, ins=[cc_in.opt()], outs=[cc_out.opt()],
                replica_groups=[list(range(N_CORES))])
            S_sb = bigpool.tile([128, 1], F32, tag="S", name="S")
            nc.sync.dma_start(out=S_sb[:], in_=cc_out[:])
            m_sb = bigpool.tile([128, 1], F32, tag="m", name="m")
            nc.vector.tensor_scalar(out=m_sb[:], in0=S_sb[:],
                                    scalar1=1.0 / (B * N * E), scalar2=1e-6,
                                    op0=AO.mult, op1=AO.add)
            r_sb = bigpool.tile([128, 1], F32, tag="r", name="r")
            nc.vector.reciprocal(r_sb[:], m_sb[:])
            a_sb = bigpool.tile([128, 1], F32, tag="a", name="a")
            nc.vector.tensor_scalar(out=a_sb[:], in0=r_sb[:], scalar1=-1.0,
                                    scalar2=None, op0=AO.mult)

            # ---- bisect state ----
            lo = []
            for b in range(BPC):
                lo.append(bigpool.tile([128, 1], F32, tag=f"lo{b}",
                                       name=f"lo{b}"))
                nc.vector.memset(lo[b][:], -W0 / 2)

            def mlp_group(b, G):
                blk, g0 = (4 * G) // 8, (4 * G) % 8
                toks = {}
                xyzw = {}
                for hl, tok_d, xyz_d in (("h", tokh_d, xyzh_d),
                                         ("l", tokl_d, xyzl_d)):
                    tt = []
                    for kc in range(4):
                        t = tokp.tile([128, 2048], F16, tag=f"tok{hl}{kc}",
                                      name=f"tok{hl}{kc}")
                        nc.sync.dma_start(
                            out=t[:],
                            in_=tok_d[b, kc * 128:(kc + 1) * 128,
                                      G * 2048:(G + 1) * 2048])
                        tt.append(t)
                    toks[hl] = tt
                xyzs = tokp.tile([9, 2048], F16, tag="xyzs", name="xyzs")
                for base, xd in ((0, xyzh_d), (3, xyzl_d), (6, xyzh_d)):
                    nc.sync.dma_start(
                        out=xyzs[base:base + 3, :],
                        in_=xd[b, :, G * 2048:(G + 1) * 2048])
                lgrp = work.tile([16, 2048], F32, tag="lgrp", name="lgrp")
                for q in range(4):
                    qs = slice(512 * q, 512 * (q + 1))
                    h_sb = []
                    TERMS = (("h", "h"), ("l", "h"), ("h", "l"))
                    for mc in range(2):
                        p_h = ps.tile([128, 512], F32, tag="h", name="p_h")
                        first = True
                        for tmv, tst in TERMS:
                            for kc in range(4):
                                nc.tensor.matmul(p_h[:],
                                                 w1_sb[tst][kc][mc][:],
                                                 toks[tmv][kc][:, qs],
                                                 start=first, stop=False)
                                first = False
                        nc.tensor.matmul(p_h[:], w1x9_sb[mc][:],
                                         xyzs[:, qs], start=False, stop=True)
                        t_h = work.tile([128, 512], MMDT, tag=f"h{mc}",
                                        name=f"h{mc}")
                        nc.scalar.activation(t_h[:], p_h[:], AF.Gelu,
                                             bias=b1_sb[mc][:], scale=1.0)
                        h_sb.append(t_h)
                    p_l2 = ps2.tile([16, 512], F32, tag="l2", name="p_l2")
                    for c in range(2):
                        nc.tensor.matmul(p_l2[:], r(w2_sb[c][:]), r(h_sb[c][:]),
                                         start=(c == 0), stop=(c == 1))
                    nc.scalar.activation(lgrp[:, qs], p_l2[:], AF.Identity,
                                         bias=b2t_sb[:])
                # scatter the group's logits into the packed layout
                half = 64 * (G % 2)
                off = b * 1024 + blk * 512
                for q in range(4):
                    nc.sync.dma_start(
                        out=logits_A[half + 16 * q:half + 16 * (q + 1),
                                     off:off + 512],
                        in_=lgrp[:, 512 * q:512 * (q + 1)])

            def finalize_logits(b):
                sl = slice(b * 1024, (b + 1) * 1024)
                nc.vector.scalar_tensor_tensor(
                    out=logits_A[:, sl], in0=dists_A[:, sl], scalar=a_sb[:],
                    in1=logits_A[:, sl], op0=AO.mult, op1=AO.add)

            def bisect_iter(b, i, t_acc, t_s, t_mid):
                # count tokens with logit > mid = lo + w; the per-(g,e)
                # partial counts are summed over g and broadcast back to
                # all 128 partitions by one m2 matmul.
                sl = slice(b * 1024, (b + 1) * 1024)
                w = W0 / (2 ** (i + 1))
                nc.vector.tensor_scalar(out=t_mid[:], in0=lo[b][:],
                                        scalar1=w, scalar2=None, op0=AO.add)
                nc.vector.tensor_scalar(out=mscr[b][:], in0=logits_A[:, sl],
                                        scalar1=t_mid[:], scalar2=0.0,
                                        op0=AO.is_gt, op1=AO.add,
                                        accum_out=t_acc[:])
                p_cnt = ps2.tile([128, 1], F32, tag="xt", name="p_cnt", bufs=1)
                nc.tensor.matmul(p_cnt[:], m2_sb[:], t_acc[:],
                                 start=True, stop=True)
                nc.vector.tensor_scalar(out=t_s[:], in0=p_cnt[:],
                                        scalar1=float(KSEL), scalar2=None,
                                        op0=AO.is_ge)
                nc.vector.scalar_tensor_tensor(
                    out=lo[b][:], in0=t_s[:], scalar=w, in1=lo[b][:],
                    op0=AO.mult, op1=AO.add)

            def bisect_both():
                accs = [work.tile([128, 1], F32, tag=f"pacc{b}",
                                  name=f"pacc{b}", bufs=2) for b in range(BPC)]
                ss = [work.tile([128, 1], F32, tag=f"sel{b}",
                                name=f"sel{b}", bufs=2) for b in range(BPC)]
                mids = [work.tile([128, 1], F32, tag=f"mid{b}",
                                  name=f"mid{b}", bufs=2) for b in range(BPC)]
                for i in range(N_ITER):
                    for b in range(BPC):
                        bisect_iter(b, i, accs[b], ss[b], mids[b])

            def emit(b):
                sl = slice(b * 1024, (b + 1) * 1024)
                sigT = sigs[b]
                dispT = work.tile([128, 1024], MMDT, tag="disp", name="disp")
                nc.vector.scalar_tensor_tensor(
                    out=dispT[:], in0=logits_A[:, sl], scalar=lo[b][:],
                    in1=sigT[:], op0=AO.is_gt, op1=AO.mult)
                nc.vector.tensor_scalar(out=dispT[:], in0=dispT[:],
                                        scalar1=DSCALE, scalar2=DFLOOR,
                                        op0=AO.mult, op1=AO.add)
                combT = work.tile([128, 1024], F32, tag="comb", name="comb")
                for hh in range(2):
                    hs = slice(hh * 512, (hh + 1) * 512)
                    p_den = ps2.tile([128, 512], F32, tag="pd", name="p_den", bufs=1)
                    nc.tensor.matmul(p_den[:], r(bd16_sb[:]), r(dispT[:, hs]),
                                     start=True, stop=True)
                    r_den = work.tile([128, 512], F32, tag="rden", name="rden")
                    rscr = work.tile([128, 512], F32, tag="rscr", name="rscr")
                    nc.vector.reciprocal_approx_accurate(r_den[:], p_den[:],
                                                         rscr[:])
                    nc.vector.tensor_tensor(out=combT[:, hs], in0=dispT[:, hs],
                                            in1=r_den[:], op=AO.mult)
                for hh in range(2):
                    hp = slice(hh * 64, (hh + 1) * 64)
                    nc.sync.dma_start(out=disp_d[hp, sl], in_=dispT[hp, :])
                    nc.sync.dma_start(out=comb_d[hp, sl], in_=combT[hp, :])
                if debug:
                    nc.sync.dma_start(out=dbg_logits_d[:, sl],
                                      in_=logits_A[:, sl])
                    nc.sync.dma_start(out=dbg_dists_d[:, sl],
                                      in_=dists_A[:, sl])

            # phase order: both MLPs stream back-to-back (DMA-paced); the
            # two bisect chains then run interleaved so each batch's PE
            # count-matmul overlaps the other batch's DVE mask pass.
            for G in range(4):
                mlp_group(0, G)
            for G in range(4):
                mlp_group(1, G)
            finalize_logits(0)
            finalize_logits(1)
            sigs = []
            for b in range(BPC):
                st = work.tile([128, 1024], F32, tag="sig", name="sig")
                nc.scalar.activation(st[:],
                                     logits_A[:, b * 1024:(b + 1) * 1024],
                                     AF.Sigmoid)
                sigs.append(st)
            bisect_both()
            emit(0)
            emit(1)

    nc.finalize()
    return nc


def _get_prog(debug=False):
    key = ("prog", debug, _MM_F32R, N_ITER)
    if key not in _prog_cache:
        _prog_cache[key] = _build(debug)
    return _prog_cache[key]


def make_in_maps(inputs):
    tokens = np.asarray(inputs["tokens"], dtype=np.float32)
    xyz = np.asarray(inputs["spatial_xyz"], dtype=np.float32)
    W1 = np.ascontiguousarray(np.asarray(inputs["W1"], dtype=np.float32))
    b1 = np.asarray(inputs["b1"], dtype=np.float32)
    W2 = np.ascontiguousarray(np.asarray(inputs["W2"], dtype=np.float32))
    b2 = np.asarray(inputs["b2"], dtype=np.float32)
    centers = np.asarray(inputs["centers"], dtype=np.float32)

    b2t = np.ascontiguousarray(b2[:, None])
    # S1[(c,g), g'*16+e] = -2*centers[e,c] if g==g'; S2[...] = 1 if g==g'
    S1 = np.zeros((24, 128), np.float32)
    S2 = np.zeros((24, 128), np.float32)
    for g in range(8):
        for c in range(3):
            S1[c * 8 + g, g * 16:(g + 1) * 16] = -2.0 * centers[:, c]
            S2[c * 8 + g, g * 16:(g + 1) * 16] = 1.0
    c2rep = np.ascontiguousarray(
        np.tile((centers * centers).sum(-1), 8)[:, None].astype(np.float32))
    bd16 = np.ascontiguousarray(
        (np.arange(128)[:, None] // 16 == np.arange(128)[None, :] // 16)
        .astype(np.float32))
    m2 = np.ascontiguousarray(
        (np.arange(128)[:, None] % 16 == np.arange(128)[None, :] % 16)
        .astype(np.float32))

    in_maps = []
    for c in range(N_CORES):
        sl = slice(BPC * c, BPC * (c + 1))
        tokT = np.ascontiguousarray(tokens[sl].transpose(0, 2, 1))
        xyzT = np.ascontiguousarray(xyz[sl].transpose(0, 2, 1))
        in_maps.append({
            "tokh": tokT.astype(np.float16),
            "tokl": (tokT - tokT.astype(np.float16).astype(np.float32))
                    .astype(np.float16),
            "xyzT": xyzT,
            "xyzh": xyzT.astype(np.float16),
            "xyzl": (xyzT - xyzT.astype(np.float16).astype(np.float32))
                    .astype(np.float16),
            "W1h": W1.astype(np.float16),
            "W1l": (W1 - W1.astype(np.float16).astype(np.float32))
                   .astype(np.float16),
            "W1x9": np.ascontiguousarray(np.concatenate([
                W1.astype(np.float16)[512:515],
                W1.astype(np.float16)[512:515],
                (W1 - W1.astype(np.float16).astype(np.float32))
                .astype(np.float16)[512:515]], axis=0)),
            "b1": b1, "W2": W2, "b2t": b2t,
            "S1": S1, "S2": S2, "c2rep": c2rep, "bd16": bd16, "m2": m2,
        })
    return in_maps


def _unscramble(x):
    # [128, 2048] (p=g*16+e, f=b*1024+blk*512+t) -> [BPC, N, E]
    return np.ascontiguousarray(
        x.reshape(8, E, BPC, 2, 512).transpose(2, 3, 0, 4, 1)
        .reshape(BPC, N, E))


def kernel(**inputs):
    from concourse.bass_utils import run_bass_kernel_spmd

    nc = _get_prog(_DEBUG)
    in_maps = make_in_maps(inputs)
    res = run_bass_kernel_spmd(nc, in_maps, list(range(N_CORES)))
    dispatch = np.concatenate(
        [_unscramble(res.results[c]["disp"]) for c in range(N_CORES)], axis=0)
    combine = np.concatenate(
        [_unscramble(res.results[c]["comb"]) for c in range(N_CORES)], axis=0)
    if _DEBUG:
        kernel._dbg = [(res.results[c]["dbg_logits"], res.results[c]["dbg_dists"])
                       for c in range(N_CORES)]
    return dispatch, combine
